# revision 1
# baseline (speedup 1.0000x reference)
"""Trainium2 Bass kernel for nn_EvolvingGNN (LSTM-evolved GCN + edge MLP).

Strategy (8 NeuronCores, full inputs in / full output out):
  - Nodes sharded 12500/core. Edges partitioned by destination core.
  - LSTM distributed: each core computes 512 of the 4096 gate rows
    (reads only its slice of W_ih/W_hh), AllGather of h each step.
  - xwd[n] = dinv[n] * (x[n] @ W) computed on the node shard, AllGathered
    into a full 256B-row table for gathers.
  - Message phase: dma_gather xwd[src] -> dma_scatter_add into agg[dst]
    (CCE add). Scatter calls must have unique indices (duplicate rows in
    one call race on read-modify-write), so edges are organised into
    "rounds" (r-th in-edge of each node) with round-robin over 4
    accumulator tables to hide the inter-round ordering latency.
  - emb = relu(dinv * (agg + xwd_self)); uv = [emb@W1a.T | emb@W1b.T]
    (one 256B row per node), AllGathered.
  - Edge MLP: gather uv[src] (u half) + uv[dst] (v half), w = ea@W1c.T+b1
    via PE matmuls on host-transposed edge_attr, logits = relu(z) . W2 + b2
    via DVE mul+reduce.
  - Gather indices are int16, so the node-table rows are bucketed in
    32768-row groups; the per-core edge order is (bucket, round, dst).
    Pads: gathers use row 0, scatters use a trash row.
"""

import numpy as np

import concourse.bacc as bacc


class _PhaseDone(Exception):
    pass


import concourse.mybir as mybir
import concourse.tile as tile
from concourse.bass_utils import run_bass_kernel_spmd
from concourse.masks import make_identity

F32 = mybir.dt.float32
I16 = mybir.dt.int16


class CFG:
    def __init__(self, N, E, T, DIN, DH, EF, NC=8, CHUNK=8192, CCH=4):
        self.N, self.E, self.T = N, E, T
        self.DIN, self.DH, self.EF = DIN, DH, EF
        self.FLAT = DIN * DH
        self.NC = NC
        assert N % NC == 0
        self.SH = N // NC                       # nodes per core
        self.TILES = -(-self.SH // 128)         # node tiles per core
        self.SHP = self.TILES * 128             # padded shard rows
        self.NTAB = NC * self.SHP               # full table rows
        self.NBUCK = -(-self.NTAB // 32768)
        self.CHUNK = CHUNK                      # gather chunk (edges)
        self.CCH = CCH                          # scatter chain tables
        self.ROW = 64                           # table row f32 (256B)
        # LSTM slicing: core k owns gate rows {g*FLAT + k*GSL + j}
        assert (4 * self.FLAT) % NC == 0
        self.GSL = self.FLAT // NC              # per-gate slice (128)
        self.KCH = self.FLAT // 128             # contraction chunks (8)


def _roundup(x, m):
    return -(-x // m) * m


def _idx_plane(idx16, tot):
    """dma_gather/scatter idx layout: [128, tot/16], idx j at [j%16, j//16],
    replicated across the 8 q7 core groups."""
    plane = idx16.reshape(tot // 16, 16).T.astype(np.int16)
    return np.ascontiguousarray(np.tile(plane, (8, 1)))


def host_prep(inputs, cfg):
    """Shard / reorder everything on the host. Returns (in_maps, struct, meta)."""
    c = cfg
    x_last = np.asarray(inputs["x"][-1], np.float32)            # [N, DIN]
    ei = np.asarray(inputs["edge_index"], np.int64)             # [2, E]
    ea = np.asarray(inputs["edge_attr"], np.float32)            # [E, EF]
    src, dst = ei[0], ei[1]

    deg = np.bincount(dst, minlength=c.N).astype(np.float32) + 1.0
    dinv = (1.0 / np.sqrt(deg)).astype(np.float32)

    rowid = (src // c.SH) * c.SHP + (src % c.SH)                # table row of src
    sbuck = (rowid >> 15).astype(np.int64)
    s16 = (rowid - (sbuck << 15)).astype(np.int64)
    ecore = dst // c.SH

    # ---- per-core (bucket, round, dst) ordering ----
    per_core = []
    maxr = np.zeros(c.NBUCK, np.int64)
    counts = [dict() for _ in range(c.NC)]                      # (b, r) -> n
    for k in range(c.NC):
        eids = np.flatnonzero(ecore == k)
        b = sbuck[eids]
        dloc = dst[eids] - k * c.SH
        o1 = np.lexsort((dloc, b))
        eids, b, dloc = eids[o1], b[o1], dloc[o1]
        newrun = np.r_[True, (b[1:] != b[:-1]) | (dloc[1:] != dloc[:-1])]
        starts = np.flatnonzero(newrun)
        runlen = np.arange(len(b)) - np.repeat(starts, np.diff(np.r_[starts, len(b)]))
        r = runlen                                              # occurrence index
        o2 = np.lexsort((dloc, r, b))
        eids, b, r, dloc = eids[o2], b[o2], r[o2], dloc[o2]
        per_core.append((eids, b, r, dloc))
        for bb in range(c.NBUCK):
            sel = b == bb
            if not sel.any():
                continue
            maxr[bb] = max(maxr[bb], int(r[sel].max()) + 1)
            rr, cnt = np.unique(r[sel], return_counts=True)
            for rv, cv in zip(rr, cnt):
                counts[k][(bb, int(rv))] = int(cv)

    # ---- universal segment sizes ----
    segsz = {}
    for bb in range(c.NBUCK):
        for rv in range(int(maxr[bb])):
            m = max((counts[k].get((bb, rv), 0) for k in range(c.NC)), default=0)
            if m > 0:
                segsz[(bb, rv)] = _roundup(m, 128)
    seg_off = {}
    bucket_rng = []                                             # (start, len) per bucket
    off = 0
    for bb in range(c.NBUCK):
        bstart = off
        for rv in range(int(maxr[bb])):
            if (bb, rv) in segsz:
                seg_off[(bb, rv)] = off
                off += segsz[(bb, rv)]
        bucket_rng.append((bstart, off - bstart))
    TOT = off
    assert TOT % 128 == 0

    # ---- chunk / scatter-piece structure (identical for all cores) ----
    # per bucket: list of (chunk_off, chunk_len); scatter pieces per chunk.
    pieces = []                                                 # (coff,clen,[(po,pl,chain)])
    piece_ctr = 0
    for bb in range(c.NBUCK):
        bstart, blen = bucket_rng[bb]
        if blen == 0:
            continue
        cuts = list(range(bstart, bstart + blen, c.CHUNK)) + [bstart + blen]
        for ci in range(len(cuts) - 1):
            coff, cend = cuts[ci], cuts[ci + 1]
            plist = []
            for rv in range(int(maxr[bb])):
                if (bb, rv) not in segsz:
                    continue
                so = seg_off[(bb, rv)]
                se = so + segsz[(bb, rv)]
                lo, hi = max(so, coff), min(se, cend)
                # dma_scatter_add breaks above 4096 idxs per call
                while lo < hi:
                    sub = min(hi - lo, 4096)
                    plist.append((lo - coff, sub, piece_ctr % c.CCH))
                    piece_ctr += 1
                    lo += sub
            pieces.append((bb, coff, cend - coff, plist))

    # ---- per-core slot arrays ----
    TRASH = c.SHP                                               # scatter/v pad row
    in_maps = []
    origs = []
    for k in range(c.NC):
        eids, b, r, dloc = per_core[k]
        # slot of each edge: seg_off + rank within segment
        seg_id = b * (int(maxr.max()) + 1) + r
        newseg = np.r_[True, seg_id[1:] != seg_id[:-1]]
        sstarts = np.flatnonzero(newseg)
        rank = np.arange(len(b)) - np.repeat(sstarts, np.diff(np.r_[sstarts, len(b)]))
        slot = np.array([seg_off[(int(bb), int(rv))] for bb, rv in
                         zip(b[newseg], r[newseg])], np.int64)
        slot = np.repeat(slot, np.diff(np.r_[sstarts, len(b)])) + rank

        u16 = np.zeros(TOT, np.int64)
        vs16 = np.full(TOT, TRASH, np.int64)
        eaT = np.zeros((c.EF + 1, TOT), np.float32)
        orig = np.full(TOT, -1, np.int64)
        u16[slot] = s16[eids]
        vs16[slot] = dloc
        eaT[: c.EF, slot] = ea[eids].T
        eaT[c.EF, slot] = 1.0
        orig[slot] = eids
        origs.append(orig)

        # node shard data
        n0 = k * c.SH
        xT = np.zeros((c.DIN, c.SHP), np.float32)
        xT[:, : c.SH] = x_last[n0 : n0 + c.SH].T
        dflat = np.ones(c.SHP, np.float32)
        dflat[: c.SH] = dinv[n0 : n0 + c.SH]
        dvt = np.ascontiguousarray(dflat.reshape(c.TILES, 128).T)  # [p,t]=dinv[t*128+p]

        # LSTM slices
        rows = np.concatenate(
            [g * c.FLAT + k * c.GSL + np.arange(c.GSL) for g in range(4)]
        )

        def wl(w):
            wt = np.asarray(w, np.float32)[rows].T              # [FLAT, 4*GSL]
            return np.ascontiguousarray(
                wt.reshape(c.KCH, 128, 4 * c.GSL).transpose(1, 0, 2).reshape(
                    128, c.KCH * 4 * c.GSL
                )
            )

        bsl = np.concatenate(
            [np.asarray(inputs["b_ih"], np.float32)[rows].reshape(4, c.GSL).T,
             np.asarray(inputs["b_hh"], np.float32)[rows].reshape(4, c.GSL).T],
            axis=1,
        )  # [GSL, 8]

        w0 = np.asarray(inputs["initial_weights"], np.float32).reshape(-1)
        w0t = np.ascontiguousarray(w0.reshape(c.KCH, 128).T)    # [128, KCH]

        W1 = np.asarray(inputs["W1"], np.float32)               # [DH, 2DH+EF]
        w1ab = np.ascontiguousarray(
            np.concatenate([W1[:, : c.DH].T, W1[:, c.DH : 2 * c.DH].T], axis=1)
        )                                                        # [DH, 2DH]
        w1c = np.concatenate(
            [W1[:, 2 * c.DH :].T, np.asarray(inputs["b1"], np.float32)[None, :]]
        )                                                        # [EF+1, DH]
        w2 = np.asarray(inputs["W2"], np.float32).reshape(-1)    # [DH]
        w2rep = np.ascontiguousarray(np.tile(w2, (128, 512 // c.DH)))

        in_maps.append({
            "xT": xT,
            "dinv": dvt,
            "wih": wl(inputs["W_ih"]),
            "whh": wl(inputs["W_hh"]),
            "bsl": np.ascontiguousarray(bsl.astype(np.float32)),
            "w0": w0t.astype(np.float32),
            "w1ab": w1ab,
            "w1c": np.ascontiguousarray(w1c),
            "w2rep": w2rep,
            "uidx": _idx_plane(u16, TOT),
            "vsidx": _idx_plane(vs16, TOT),
            "eaT": np.ascontiguousarray(eaT),
        })

    struct = {
        "TOT": TOT,
        "pieces": pieces,
        "b2": float(np.asarray(inputs["b2"], np.float32).reshape(-1)[0]),
    }
    return in_maps, struct, origs


def build(cfg, struct):
    import os
    NPH = int(os.environ.get("KPHASES", "9"))
    c = cfg
    TOT = struct["TOT"]
    nc = bacc.Bacc("TRN2", target_bir_lowering=False, debug=False,
                   num_devices=c.NC)

    # ---------- I/O ----------
    xT_h = nc.dram_tensor("xT", [c.DIN, c.SHP], F32, kind="ExternalInput")
    dinv_h = nc.dram_tensor("dinv", [128, c.TILES], F32, kind="ExternalInput")
    wih_h = nc.dram_tensor("wih", [128, c.KCH * 4 * c.GSL], F32, kind="ExternalInput")
    whh_h = nc.dram_tensor("whh", [128, c.KCH * 4 * c.GSL], F32, kind="ExternalInput")
    bsl_h = nc.dram_tensor("bsl", [c.GSL, 8], F32, kind="ExternalInput")
    w0_h = nc.dram_tensor("w0", [128, c.KCH], F32, kind="ExternalInput")
    w1ab_h = nc.dram_tensor("w1ab", [c.DH, 2 * c.DH], F32, kind="ExternalInput")
    w1c_h = nc.dram_tensor("w1c", [c.EF + 1, c.DH], F32, kind="ExternalInput")
    w2rep_h = nc.dram_tensor("w2rep", [128, 512], F32, kind="ExternalInput")
    uidx_h = nc.dram_tensor("uidx", [128, TOT // 16], I16, kind="ExternalInput")
    vsidx_h = nc.dram_tensor("vsidx", [128, TOT // 16], I16, kind="ExternalInput")
    eaT_h = nc.dram_tensor("eaT", [c.EF + 1, TOT], F32, kind="ExternalInput")

    logits_h = nc.dram_tensor("logits", [128, TOT // 128], F32, kind="ExternalOutput")
    # zero-initialised accumulator tables (ExternalOutput => donated zeros)
    aggs = [nc.dram_tensor(f"agg{i}", [c.SHP + 128, c.ROW], F32, kind="ExternalOutput")
            for i in range(c.CCH)]
    uv_own = nc.dram_tensor("uv_own", [c.SHP + 128, c.ROW], F32, kind="ExternalOutput")

    # internal DRAM
    xwd_own = nc.dram_tensor("xwd_own", [c.SHP, c.ROW], F32)
    xwd_full = nc.dram_tensor("xwd_full", [c.NTAB, c.ROW], F32)
    uv_shard = nc.dram_tensor("uv_shard", [c.SHP, c.ROW], F32)
    uv_full = nc.dram_tensor("uv_full", [c.NTAB, c.ROW], F32)
    hb_in = nc.dram_tensor("hb_in", [128, 1], F32)
    hb_out = nc.dram_tensor("hb_out", [c.FLAT, 1], F32)

    groups = [list(range(c.NC))]

    with tile.TileContext(nc) as tc:
        with (
            tc.tile_pool(name="persist", bufs=1) as pp,
            tc.tile_pool(name="psum_ls", bufs=2, space="PSUM") as ps_ls,
        ):
            try:
                # ---------- persistent small tiles ----------
                ident = pp.tile([128, 128], F32)
                make_identity(nc, ident[:])
                w1ab_sb = pp.tile([c.DH, 2 * c.DH], F32)
                nc.sync.dma_start(w1ab_sb[:], w1ab_h[:])
                w1c_sb = pp.tile([c.EF + 1, c.DH], F32)
                nc.sync.dma_start(w1c_sb[:], w1c_h[:])
                w2_sb = pp.tile([128, 512], F32)
                nc.sync.dma_start(w2_sb[:], w2rep_h[:])
                dinv_sb = pp.tile([128, c.TILES], F32)
                nc.sync.dma_start(dinv_sb[:], dinv_h[:])
                xwd_sb = pp.tile([128, c.TILES, c.DH], F32)  # persists to post-agg
                W_sb = pp.tile([c.DIN, c.DH], F32)           # evolved GCN weight

                # ---------- phase 0: distributed LSTM ----------
                with tc.tile_pool(name="lstm", bufs=1) as lp:
                    wih_sb = lp.tile([128, c.KCH * 4 * c.GSL], F32)
                    whh_sb = lp.tile([128, c.KCH * 4 * c.GSL], F32)
                    nc.sync.dma_start(wih_sb[:], wih_h[:])
                    nc.sync.dma_start(whh_sb[:], whh_h[:])
                    bsl_sb = lp.tile([c.GSL, 8], F32)
                    nc.sync.dma_start(bsl_sb[:], bsl_h[:])
                    bsum = lp.tile([c.GSL, 4], F32)
                    nc.vector.tensor_tensor(bsum[:], bsl_sb[:, 0:4], bsl_sb[:, 4:8],
                                            op=mybir.AluOpType.add)
                    inp = lp.tile([128, c.KCH], F32)
                    nc.sync.dma_start(inp[:], w0_h[:])
                    cstate = lp.tile([c.GSL, 1], F32)
                    gsb = lp.tile([c.GSL, 4], F32)
                    ifgo = lp.tile([c.GSL, 4], F32)
                    tmp = lp.tile([c.GSL, 2], F32)

                    wv = wih_sb[:].rearrange("p (c n) -> p c n", c=c.KCH)
                    wsumv = whh_sb[:].rearrange("p (c n) -> p c n", c=c.KCH)

                    for step in range(c.T):
                        wmat = wv if step == 0 else wsumv
                        gp = ps_ls.tile([c.GSL, 4], F32, tag="gates")
                        for g in range(4):
                            for kc in range(c.KCH):
                                nc.tensor.matmul(
                                    gp[:, g : g + 1],
                                    wmat[:, kc, g * c.GSL : (g + 1) * c.GSL],
                                    inp[:, kc : kc + 1],
                                    start=(kc == 0),
                                    stop=(kc == c.KCH - 1),
                                )
                        if step == 0:
                            # wsum = wih + whh (for steps 2..T), overwrite whh
                            nc.vector.tensor_tensor(whh_sb[:], wih_sb[:], whh_sb[:],
                                                    op=mybir.AluOpType.add)
                        nc.vector.tensor_tensor(gsb[:], gp[:], bsum[:],
                                                op=mybir.AluOpType.add)
                        Sig = mybir.ActivationFunctionType.Sigmoid
                        Tanh = mybir.ActivationFunctionType.Tanh
                        nc.scalar.activation(ifgo[:, 0:1], gsb[:, 0:1], Sig)
                        nc.scalar.activation(ifgo[:, 1:2], gsb[:, 1:2], Sig)
                        nc.scalar.activation(ifgo[:, 2:3], gsb[:, 2:3], Tanh)
                        nc.scalar.activation(ifgo[:, 3:4], gsb[:, 3:4], Sig)
                        # c' = f*c + i*g ; h' = o * tanh(c')
                        nc.vector.tensor_tensor(tmp[:, 0:1], ifgo[:, 0:1], ifgo[:, 2:3],
                                                op=mybir.AluOpType.mult)
                        if step == 0:
                            nc.vector.tensor_copy(cstate[:], tmp[:, 0:1])
                        else:
                            nc.vector.tensor_tensor(tmp[:, 1:2], ifgo[:, 1:2], cstate[:],
                                                    op=mybir.AluOpType.mult)
                            nc.vector.tensor_tensor(cstate[:], tmp[:, 0:1], tmp[:, 1:2],
                                                    op=mybir.AluOpType.add)
                        nc.scalar.activation(tmp[:, 0:1], cstate[:], Tanh)
                        h2 = tmp[:, 1:2]
                        nc.vector.tensor_tensor(h2, ifgo[:, 3:4], tmp[:, 0:1],
                                                op=mybir.AluOpType.mult)
                        # allgather h2 -> full h
                        nc.gpsimd.dma_start(hb_in[:, :], h2)
                        nc.gpsimd.collective_compute(
                            "AllGather", mybir.AluOpType.bypass,
                            replica_groups=groups,
                            ins=[hb_in[:, :].opt()],
                            outs=[hb_out[:, :].opt()],
                        )
                        if step < c.T - 1:
                            nc.sync.dma_start(
                                inp[:], hb_out[:, 0].rearrange("(c p) -> p c", p=128)
                            )
                        else:
                            nc.sync.dma_start(
                                W_sb[:],
                                hb_out[:, 0].rearrange("(a b) -> a b", a=c.DIN),
                            )

                # ---------- phase B: xwd = dinv * (x @ W) ----------
                if NPH < 2:
                    raise _PhaseDone()
                with (
                    tc.tile_pool(name="xw", bufs=3) as xp,
                    tc.tile_pool(name="psum_xw", bufs=4, space="PSUM") as ps_xw,
                ):
                    xT_sb = xp.tile([c.DIN, c.SHP], F32, tag="xT")
                    nc.sync.dma_start(xT_sb[:], xT_h[:])
                    for t in range(c.TILES):
                        pxw = ps_xw.tile([128, c.DH], F32, tag="pxw")
                        nc.tensor.matmul(pxw[:], xT_sb[:, t * 128 : (t + 1) * 128],
                                         W_sb[:], start=True, stop=True)
                        nc.vector.tensor_scalar(
                            xwd_sb[:, t, :], pxw[:], dinv_sb[:, t : t + 1], None,
                            op0=mybir.AluOpType.mult,
                        )
                        nc.sync.dma_start(
                            xwd_own[t * 128 : (t + 1) * 128, 0 : c.DH],
                            xwd_sb[:, t, :],
                        )

                if NPH < 3:
                    raise _PhaseDone()
                tc.strict_bb_all_engine_barrier()
                nc.gpsimd.collective_compute(
                    "AllGather", mybir.AluOpType.bypass,
                    replica_groups=groups,
                    ins=[xwd_own[:, :].opt()],
                    outs=[xwd_full[:, :].opt()],
                )
                tc.strict_bb_all_engine_barrier()

                # ---------- phase 1: gather msgs + scatter-add ----------
                if NPH < 4:
                    raise _PhaseDone()
                with tc.tile_pool(name="p1", bufs=3) as p1:
                    for bb, coff, clen, plist in struct["pieces"]:
                        ui = p1.tile([128, c.CHUNK // 16], I16, tag="ui")
                        vi = p1.tile([128, c.CHUNK // 16], I16, tag="vi")
                        nc.sync.dma_start(ui[:, : clen // 16],
                                          uidx_h[:, coff // 16 : (coff + clen) // 16])
                        nc.sync.dma_start(vi[:, : clen // 16],
                                          vsidx_h[:, coff // 16 : (coff + clen) // 16])
                        msg = p1.tile([128, c.CHUNK // 128, c.ROW], F32, tag="msg")
                        nc.gpsimd.dma_gather(
                            msg[:, : clen // 128, :],
                            xwd_full[bb * 32768 :, :],
                            ui[:, : clen // 16],
                            clen, clen, c.ROW, single_packet=False,
                        )
                        for po, pl, chain in plist:
                            nc.gpsimd.dma_scatter_add(
                                aggs[chain][:, :],
                                msg[:, po // 128 : (po + pl) // 128, :],
                                vi[:, po // 16 : (po + pl) // 16],
                                pl, pl, c.ROW, single_packet=False,
                            )

                tc.strict_bb_all_engine_barrier()

                # ---------- phase 2: emb, uv tables ----------
                if NPH < 5:
                    raise _PhaseDone()
                with (
                    tc.tile_pool(name="p2", bufs=3) as p2,
                    tc.tile_pool(name="psum_t", bufs=2, space="PSUM") as ps_t,
                    tc.tile_pool(name="psum_uv", bufs=2, space="PSUM") as ps_uv,
                ):
                    for t in range(c.TILES):
                        r0, r1 = t * 128, (t + 1) * 128
                        ag = [p2.tile([128, c.ROW], F32, tag=f"ag{i}", name=f"ag{i}")
                              for i in range(c.CCH)]
                        for i in range(c.CCH):
                            nc.sync.dma_start(ag[i][:], aggs[i][r0:r1, :])
                        s0 = p2.tile([128, c.DH], F32, tag="s0")
                        s1 = p2.tile([128, c.DH], F32, tag="s1")
                        nc.vector.tensor_tensor(s0[:], ag[0][:, : c.DH], ag[1][:, : c.DH],
                                                op=mybir.AluOpType.add)
                        nc.vector.tensor_tensor(s1[:], ag[2][:, : c.DH], ag[3][:, : c.DH],
                                                op=mybir.AluOpType.add)
                        nc.vector.tensor_tensor(s0[:], s0[:], s1[:],
                                                op=mybir.AluOpType.add)
                        nc.vector.tensor_tensor(s0[:], s0[:], xwd_sb[:, t, :],
                                                op=mybir.AluOpType.add)
                        emb = p2.tile([128, c.DH], F32, tag="emb")
                        nc.scalar.activation(emb[:], s0[:],
                                             mybir.ActivationFunctionType.Relu,
                                             scale=dinv_sb[:, t : t + 1])
                        pt = ps_t.tile([c.DH, 128], F32, tag="pt")
                        nc.tensor.transpose(pt[:], emb[:], ident[:])
                        embT = p2.tile([c.DH, 128], F32, tag="embT")
                        nc.vector.tensor_copy(embT[:], pt[:])
                        puv = ps_uv.tile([128, 2 * c.DH], F32, tag="puv")
                        nc.tensor.matmul(puv[:], embT[:], w1ab_sb[:],
                                         start=True, stop=True)
                        uvt = p2.tile([128, c.ROW], F32, tag="uvt")
                        nc.vector.tensor_copy(uvt[:, : 2 * c.DH], puv[:])
                        nc.sync.dma_start(uv_own[r0:r1, :], uvt[:])
                        nc.sync.dma_start(uv_shard[r0:r1, :], uvt[:])

                tc.strict_bb_all_engine_barrier()
                nc.gpsimd.collective_compute(
                    "AllGather", mybir.AluOpType.bypass,
                    replica_groups=groups,
                    ins=[uv_shard[:, :].opt()],
                    outs=[uv_full[:, :].opt()],
                )
                tc.strict_bb_all_engine_barrier()

                # ---------- phase 3: edge MLP ----------
                if NPH < 6:
                    raise _PhaseDone()
                b2 = struct["b2"]
                with (
                    tc.tile_pool(name="p3", bufs=2) as p3,
                    tc.tile_pool(name="psum_w", bufs=4, space="PSUM") as ps_w,
                ):
                    for bb, coff, clen, _pl in struct["pieces"]:
                        ui = p3.tile([128, c.CHUNK // 16], I16, tag="ui3")
                        vi = p3.tile([128, c.CHUNK // 16], I16, tag="vi3")
                        nc.sync.dma_start(ui[:, : clen // 16],
                                          uidx_h[:, coff // 16 : (coff + clen) // 16])
                        nc.sync.dma_start(vi[:, : clen // 16],
                                          vsidx_h[:, coff // 16 : (coff + clen) // 16])
                        ug = p3.tile([128, c.CHUNK // 128, c.ROW], F32, tag="ug")
                        vg = p3.tile([128, c.CHUNK // 128, c.ROW], F32, tag="vg")
                        nc.gpsimd.dma_gather(
                            ug[:, : clen // 128, :], uv_full[bb * 32768 :, :],
                            ui[:, : clen // 16], clen, clen, c.ROW,
                            single_packet=False,
                        )
                        nc.gpsimd.dma_gather(
                            vg[:, : clen // 128, :], uv_own[:, :],
                            vi[:, : clen // 16], clen, clen, c.ROW,
                            single_packet=False,
                        )
                        eat = p3.tile([c.EF + 1, c.CHUNK], F32, tag="eat")
                        nc.sync.dma_start(eat[:, :clen],
                                          eaT_h[:, coff : coff + clen])
                        lg = p3.tile([128, c.CHUNK // 128], F32, tag="lg")
                        ngrp = -(-clen // 2048)
                        for g in range(ngrp):
                            e0 = g * 2048
                            gl = min(2048, clen - e0)               # multiple of 128
                            nbk = gl // 128
                            pw = ps_w.tile([128, 512], F32, tag="pw")
                            for e in range(nbk):
                                nc.tensor.matmul(
                                    pw[:, e * c.DH : (e + 1) * c.DH],
                                    eat[:, e0 + e * 128 : e0 + (e + 1) * 128],
                                    w1c_sb[:], start=True, stop=True,
                                )
                            z = p3.tile([128, 16, c.DH], F32, tag="z")
                            blk = slice(e0 // 128, e0 // 128 + nbk)
                            nc.vector.tensor_tensor(
                                z[:, :nbk, :], ug[:, blk, : c.DH],
                                vg[:, blk, c.DH : 2 * c.DH], op=mybir.AluOpType.add,
                            )
                            nc.vector.tensor_tensor(
                                z[:].rearrange("p a b -> p (a b)")[:, : nbk * c.DH],
                                z[:].rearrange("p a b -> p (a b)")[:, : nbk * c.DH],
                                pw[:, : nbk * c.DH],
                                op=mybir.AluOpType.add,
                            )
                            nc.scalar.activation(
                                z[:, :nbk, :], z[:, :nbk, :],
                                mybir.ActivationFunctionType.Relu,
                            )
                            nc.vector.tensor_tensor(
                                z[:, :nbk, :], z[:, :nbk, :],
                                w2_sb[:].rearrange("p (a b) -> p a b", b=c.DH)[:, :nbk, :],
                                op=mybir.AluOpType.mult,
                            )
                            nc.vector.tensor_reduce(
                                lg[:, blk], z[:, :nbk, :],
                                axis=mybir.AxisListType.X, op=mybir.AluOpType.add,
                            )
                        if b2 != 0.0:
                            nc.vector.tensor_scalar_add(lg[:, : clen // 128],
                                                        lg[:, : clen // 128], b2)
                        nc.sync.dma_start(
                            logits_h[:, coff // 128 : (coff + clen) // 128],
                            lg[:, : clen // 128],
                        )

            except _PhaseDone:
                pass
    nc.compile()
    return nc


_BUILD_CACHE = {}


def _kernel_impl(inputs, cfg):
    in_maps, struct, origs = host_prep(inputs, cfg)
    key = (cfg.N, cfg.E, struct["TOT"], str(struct["pieces"]), struct["b2"])
    if key not in _BUILD_CACHE:
        _BUILD_CACHE.clear()
        _BUILD_CACHE[key] = build(cfg, struct)
    nc = _BUILD_CACHE[key]
    res = run_bass_kernel_spmd(nc, in_maps, list(range(cfg.NC)))
    out = np.empty(cfg.E, np.float32)
    for k in range(cfg.NC):
        lg = res.results[k]["logits"]           # [128, TOT/128]
        flat = lg.T.reshape(-1)
        orig = origs[k]
        valid = orig >= 0
        out[orig[valid]] = flat[valid]
    return out


def kernel(**inputs):
    cfg = CFG(N=100000, E=1_600_000, T=5, DIN=32, DH=32, EF=16)
    return _kernel_impl(inputs, cfg)



# revision 3
# speedup vs baseline: 3.3701x; 3.3701x over previous
"""Trainium2 Bass kernel for nn_EvolvingGNN (LSTM-evolved GCN + edge MLP).

Strategy (8 NeuronCores, full inputs in / full output out):
  - Nodes sharded 12500/core. Edges partitioned by destination core.
  - LSTM distributed: each core computes 512 of the 4096 gate rows
    (reads only its slice of W_ih/W_hh), AllGather of h each step.
  - xwd[n] = dinv[n] * (x[n] @ W) computed on the node shard, AllGathered
    into a full 256B-row table for gathers.
  - Message phase: dma_gather xwd[src] -> dma_scatter_add into agg[dst]
    (CCE add). Scatter calls must have unique indices (duplicate rows in
    one call race on read-modify-write), so edges are organised into
    "rounds" (r-th in-edge of each node) with round-robin over 4
    accumulator tables to hide the inter-round ordering latency.
  - emb = relu(dinv * (agg + xwd_self)); uv = [emb@W1a.T | emb@W1b.T]
    (one 256B row per node), AllGathered.
  - Edge MLP: gather uv[src] (u half) + uv[dst] (v half), w = ea@W1c.T+b1
    via PE matmuls on host-transposed edge_attr, logits = relu(z) . W2 + b2
    via DVE mul+reduce.
  - Gather indices are int16, so the node-table rows are bucketed in
    32768-row groups; the per-core edge order is (bucket, round, dst).
    Pads: gathers use row 0, scatters use a trash row.

Wall-clock optimisations (the axon tunnel moves ~60MB/s, so host->device
bytes dominate the end-to-end time):
  - Accumulator tables and uv_own are Internal DRAM zeroed on device
    (previously ExternalOutputs: ~13MB/core of donated zeros uploaded and
    ~16MB/core of unused outputs downloaded per call).
  - Big payloads (edge features, x, LSTM weights) travel as bfloat16;
    matmuls run bf16 x bf16 -> f32 PSUM.
  - Gather/scatter index planes are sent as the 16-partition master copy
    and replicated to the 128-partition layout on device (8x fewer bytes).
  - host_prep is vectorised: one combined-key argsort pipeline over all
    edges instead of per-core lexsorts.
"""

import numpy as np
import ml_dtypes

import concourse.bacc as bacc
import concourse.mybir as mybir
import concourse.tile as tile
from concourse.bass_utils import run_bass_kernel_spmd
from concourse.masks import make_identity

F32 = mybir.dt.float32
BF16 = mybir.dt.bfloat16
I16 = mybir.dt.int16
NPBF16 = ml_dtypes.bfloat16


class CFG:
    def __init__(self, N, E, T, DIN, DH, EF, NC=8, CHUNK=8192, CCH=4):
        self.N, self.E, self.T = N, E, T
        self.DIN, self.DH, self.EF = DIN, DH, EF
        self.FLAT = DIN * DH
        self.NC = NC
        assert N % NC == 0
        self.SH = N // NC                       # nodes per core
        self.TILES = -(-self.SH // 128)         # node tiles per core
        self.SHP = self.TILES * 128             # padded shard rows
        self.NTAB = NC * self.SHP               # full table rows
        self.NBUCK = -(-self.NTAB // 32768)
        self.CHUNK = CHUNK                      # gather chunk (edges)
        self.CCH = CCH                          # scatter chain tables
        self.ROW = 64                           # table row f32 (256B)
        # LSTM slicing: core k owns gate rows {g*FLAT + k*GSL + j}
        assert (4 * self.FLAT) % NC == 0
        self.GSL = self.FLAT // NC              # per-gate slice (128)
        self.KCH = self.FLAT // 128             # contraction chunks (8)


def _roundup(x, m):
    return -(-x // m) * m


def host_prep(inputs, cfg):
    """Shard / reorder everything on the host. Returns (in_maps, struct, origs)."""
    c = cfg
    x_last = np.asarray(inputs["x"][-1], np.float32)            # [N, DIN]
    ei = np.asarray(inputs["edge_index"])                       # [2, E]
    ea = np.asarray(inputs["edge_attr"], np.float32)            # [E, EF]
    src = ei[0].astype(np.int64)
    dst = ei[1].astype(np.int64)

    deg = np.bincount(dst, minlength=c.N).astype(np.float32) + 1.0
    dinv = (1.0 / np.sqrt(deg)).astype(np.float32)

    rowid = (src // c.SH) * c.SHP + (src % c.SH)                # table row of src
    sbuck = rowid >> 15
    s16 = rowid & 32767
    ecore = dst // c.SH
    dloc = dst - ecore * c.SH

    # ---- global (core, bucket, round, dst) ordering ----
    key1 = (ecore * c.NBUCK + sbuck) * c.SH + dloc
    o1 = np.argsort(key1, kind="stable")
    k1 = key1[o1]
    newrun = np.empty(c.E, bool)
    newrun[0] = True
    np.not_equal(k1[1:], k1[:-1], out=newrun[1:])
    starts = np.flatnonzero(newrun)
    r1 = np.arange(c.E) - np.repeat(starts, np.diff(np.r_[starts, c.E]))
    MAXR = int(r1.max()) + 1
    key2 = ((ecore[o1] * c.NBUCK + sbuck[o1]) * MAXR + r1) * c.SH + dloc[o1]
    o2 = np.argsort(key2, kind="stable")
    eid2 = o1[o2]
    ec2, b2v, r2, d2 = ecore[eid2], sbuck[eid2], r1[o2], dloc[eid2]

    # ---- universal segment sizes: max count over cores per (bucket, round) ----
    ckey = (ec2 * c.NBUCK + b2v) * MAXR + r2
    cnt = np.bincount(ckey, minlength=c.NC * c.NBUCK * MAXR).reshape(
        c.NC, c.NBUCK, MAXR)
    segmax = cnt.max(axis=0)                                    # [NBUCK, MAXR]
    segsz = np.where(segmax > 0, ((segmax + 127) // 128) * 128, 0).astype(np.int64)
    seg_off = np.concatenate([[0], np.cumsum(segsz.reshape(-1))])[:-1].reshape(
        c.NBUCK, MAXR)
    TOT = int(segsz.sum())
    assert TOT % c.CHUNK == 0 or TOT % 128 == 0

    # ---- per-edge slot ----
    newseg = np.empty(c.E, bool)
    newseg[0] = True
    np.not_equal(ckey[1:], ckey[:-1], out=newseg[1:])
    sstarts = np.flatnonzero(newseg)
    rank = np.arange(c.E) - np.repeat(sstarts, np.diff(np.r_[sstarts, c.E]))
    slot = seg_off[b2v, r2] + rank                              # [0, TOT) per core
    gslot = ec2 * TOT + slot

    # ---- chunk / scatter-piece structure (identical for all cores) ----
    blen = segsz.sum(axis=1)                                    # per bucket
    bstarts = np.concatenate([[0], np.cumsum(blen)])
    pieces = []                                                 # (bb,coff,clen,[(po,pl,chain)])
    piece_ctr = 0
    for bb in range(c.NBUCK):
        bstart, bl = int(bstarts[bb]), int(blen[bb])
        if bl == 0:
            continue
        cuts = list(range(bstart, bstart + bl, c.CHUNK)) + [bstart + bl]
        for ci in range(len(cuts) - 1):
            coff, cend = cuts[ci], cuts[ci + 1]
            plist = []
            for rv in range(MAXR):
                if segsz[bb, rv] == 0:
                    continue
                so = int(seg_off[bb, rv])
                se = so + int(segsz[bb, rv])
                lo, hi = max(so, coff), min(se, cend)
                # dma_scatter_add breaks above 4096 idxs per call
                while lo < hi:
                    sub = min(hi - lo, 4096)
                    plist.append((lo - coff, sub, piece_ctr % c.CCH))
                    piece_ctr += 1
                    lo += sub
            pieces.append((bb, coff, cend - coff, plist))

    # ---- global slot-order tables ----
    TRASH = c.SHP                                               # scatter/v pad row
    NT = c.NC * TOT
    u16_all = np.zeros(NT, np.int16)
    u16_all[gslot] = s16[eid2].astype(np.int16)
    vs_all = np.full(NT, TRASH, np.int16)
    vs_all[gslot] = d2.astype(np.int16)
    orig_all = np.full(NT, -1, np.int64)
    orig_all[gslot] = eid2

    ea16 = ea.astype(NPBF16)
    ea_rows = np.zeros((NT, c.EF + 1), NPBF16)
    ea_rows[gslot, : c.EF] = ea16[eid2]
    ea_rows[gslot, c.EF] = 1.0

    xlT16 = x_last.T.astype(NPBF16)                             # [DIN, N]

    W1 = np.asarray(inputs["W1"], np.float32)                   # [DH, 2DH+EF]
    w1ab = np.ascontiguousarray(
        np.concatenate([W1[:, : c.DH].T, W1[:, c.DH : 2 * c.DH].T], axis=1))
    w1c = np.ascontiguousarray(np.concatenate(
        [W1[:, 2 * c.DH :].T, np.asarray(inputs["b1"], np.float32)[None, :]]
    ).astype(NPBF16))                                           # [EF+1, DH]
    w2 = np.asarray(inputs["W2"], np.float32).reshape(-1)       # [DH]
    w2row = np.ascontiguousarray(np.tile(w2, 512 // c.DH)[None, :])  # [1, 512]
    w0 = np.asarray(inputs["initial_weights"], np.float32).reshape(-1)
    w0t = np.ascontiguousarray(w0.reshape(c.KCH, 128).T.astype(NPBF16))
    W_ih = np.asarray(inputs["W_ih"], np.float32)
    W_hh = np.asarray(inputs["W_hh"], np.float32)
    b_ih = np.asarray(inputs["b_ih"], np.float32)
    b_hh = np.asarray(inputs["b_hh"], np.float32)

    in_maps = []
    for k in range(c.NC):
        sl = slice(k * TOT, (k + 1) * TOT)
        n0 = k * c.SH

        xT = np.zeros((c.DIN, c.SHP), NPBF16)
        xT[:, : c.SH] = xlT16[:, n0 : n0 + c.SH]
        dflat = np.ones(c.SHP, np.float32)
        dflat[: c.SH] = dinv[n0 : n0 + c.SH]
        dvt = np.ascontiguousarray(dflat.reshape(c.TILES, 128).T)

        rows = np.concatenate(
            [g * c.FLAT + k * c.GSL + np.arange(c.GSL) for g in range(4)])

        def wl(w):
            wt = w[rows].T                                      # [FLAT, 4*GSL]
            return np.ascontiguousarray(
                wt.reshape(c.KCH, 128, 4 * c.GSL).transpose(1, 0, 2).reshape(
                    128, c.KCH * 4 * c.GSL).astype(NPBF16))

        bsl = np.concatenate(
            [b_ih[rows].reshape(4, c.GSL).T, b_hh[rows].reshape(4, c.GSL).T],
            axis=1)                                             # [GSL, 8]

        in_maps.append({
            "xT": xT,
            "dinv": dvt,
            "wih": wl(W_ih),
            "whh": wl(W_hh),
            "bsl": np.ascontiguousarray(bsl),
            "w0": w0t,
            "w1ab": w1ab,
            "w1c": w1c,
            "w2row": w2row,
            "uidx": np.ascontiguousarray(u16_all[sl].reshape(TOT // 16, 16).T),
            "vsidx": np.ascontiguousarray(vs_all[sl].reshape(TOT // 16, 16).T),
            "eaT": np.ascontiguousarray(ea_rows[sl].T),         # [EF+1, TOT] bf16
        })

    struct = {
        "TOT": TOT,
        "pieces": pieces,
        "b2": float(np.asarray(inputs["b2"], np.float32).reshape(-1)[0]),
    }
    return in_maps, struct, orig_all


def build(cfg, struct):
    c = cfg
    TOT = struct["TOT"]
    nc = bacc.Bacc("TRN2", target_bir_lowering=False, debug=False,
                   num_devices=c.NC)

    # ---------- I/O ----------
    xT_h = nc.dram_tensor("xT", [c.DIN, c.SHP], BF16, kind="ExternalInput")
    dinv_h = nc.dram_tensor("dinv", [128, c.TILES], F32, kind="ExternalInput")
    wih_h = nc.dram_tensor("wih", [128, c.KCH * 4 * c.GSL], BF16, kind="ExternalInput")
    whh_h = nc.dram_tensor("whh", [128, c.KCH * 4 * c.GSL], BF16, kind="ExternalInput")
    bsl_h = nc.dram_tensor("bsl", [c.GSL, 8], F32, kind="ExternalInput")
    w0_h = nc.dram_tensor("w0", [128, c.KCH], BF16, kind="ExternalInput")
    w1ab_h = nc.dram_tensor("w1ab", [c.DH, 2 * c.DH], F32, kind="ExternalInput")
    w1c_h = nc.dram_tensor("w1c", [c.EF + 1, c.DH], BF16, kind="ExternalInput")
    w2row_h = nc.dram_tensor("w2row", [1, 512], F32, kind="ExternalInput")
    uidx_h = nc.dram_tensor("uidx", [16, TOT // 16], I16, kind="ExternalInput")
    vsidx_h = nc.dram_tensor("vsidx", [16, TOT // 16], I16, kind="ExternalInput")
    eaT_h = nc.dram_tensor("eaT", [c.EF + 1, TOT], BF16, kind="ExternalInput")

    logits_h = nc.dram_tensor("logits", [128, TOT // 128], F32, kind="ExternalOutput")
    # internal accumulator tables, zeroed on device before the scatter phase
    aggs = [nc.dram_tensor(f"agg{i}", [c.SHP + 128, c.ROW], F32)
            for i in range(c.CCH)]
    uv_own = nc.dram_tensor("uv_own", [c.SHP + 128, c.ROW], F32)

    # internal DRAM
    xwd_own = nc.dram_tensor("xwd_own", [c.SHP, c.ROW], F32)
    xwd_full = nc.dram_tensor("xwd_full", [c.NTAB, c.ROW], F32, addr_space="Shared")
    uv_shard = nc.dram_tensor("uv_shard", [c.SHP, c.ROW], F32)
    uv_full = nc.dram_tensor("uv_full", [c.NTAB, c.ROW], F32, addr_space="Shared")
    hb_in = nc.dram_tensor("hb_in", [128, 1], F32)
    hb_out = nc.dram_tensor("hb_out", [c.FLAT, 1], F32)

    groups = [list(range(c.NC))]

    with tile.TileContext(nc) as tc:
        with (
            tc.tile_pool(name="persist", bufs=1) as pp,
            tc.tile_pool(name="psum_ls", bufs=2, space="PSUM") as ps_ls,
        ):
            # ---------- persistent small tiles ----------
            ident = pp.tile([128, 128], F32)
            make_identity(nc, ident[:])
            w1ab_sb = pp.tile([c.DH, 2 * c.DH], F32)
            nc.sync.dma_start(w1ab_sb[:], w1ab_h[:])
            w1c_sb = pp.tile([c.EF + 1, c.DH], BF16)
            nc.sync.dma_start(w1c_sb[:], w1c_h[:])
            dinv_sb = pp.tile([128, c.TILES], F32)
            nc.sync.dma_start(dinv_sb[:], dinv_h[:])
            xwd_sb = pp.tile([128, c.TILES, c.DH], F32)  # persists to post-agg
            W_sb = pp.tile([c.DIN, c.DH], BF16)          # evolved GCN weight

            # w2 broadcast [1,512] -> [128,512] via K=1 matmul with ones
            w2r_sb = pp.tile([1, 512], F32)
            nc.sync.dma_start(w2r_sb[:], w2row_h[:])
            ones1 = pp.tile([1, 128], F32)
            nc.vector.memset(ones1[:], 1.0)
            w2_sb = pp.tile([128, 512], F32)
            pw2 = ps_ls.tile([128, 512], F32, tag="w2bc")
            nc.tensor.matmul(pw2[:], ones1[:], w2r_sb[:], start=True, stop=True)
            nc.vector.tensor_copy(w2_sb[:], pw2[:])

            # ---------- zero the accumulator tables (device-side) ----------
            zt = pp.tile([128, 16, c.ROW], F32)
            nc.vector.memset(zt[:], 0.0)
            ntile = (c.SHP + 128) // 128
            for t in aggs:
                av = t[:, :].rearrange("(x p) c -> p x c", p=128)
                for x0 in range(0, ntile, 16):
                    xl = min(16, ntile - x0)
                    nc.sync.dma_start(av[:, x0 : x0 + xl, :], zt[:, :xl, :])
            nc.sync.dma_start(uv_own[c.SHP : c.SHP + 128, :], zt[:, 0, :])

            # ---------- phase 0: distributed LSTM ----------
            with tc.tile_pool(name="lstm", bufs=1) as lp:
                wih_sb = lp.tile([128, c.KCH * 4 * c.GSL], BF16)
                whh_sb = lp.tile([128, c.KCH * 4 * c.GSL], BF16)
                nc.sync.dma_start(wih_sb[:], wih_h[:])
                nc.sync.dma_start(whh_sb[:], whh_h[:])
                bsl_sb = lp.tile([c.GSL, 8], F32)
                nc.sync.dma_start(bsl_sb[:], bsl_h[:])
                bsum = lp.tile([c.GSL, 4], F32)
                nc.vector.tensor_tensor(bsum[:], bsl_sb[:, 0:4], bsl_sb[:, 4:8],
                                        op=mybir.AluOpType.add)
                inp = lp.tile([128, c.KCH], BF16)
                nc.sync.dma_start(inp[:], w0_h[:])
                inpf = lp.tile([128, c.KCH], F32)
                cstate = lp.tile([c.GSL, 1], F32)
                gsb = lp.tile([c.GSL, 4], F32)
                ifgo = lp.tile([c.GSL, 4], F32)
                tmp = lp.tile([c.GSL, 2], F32)
                Wf = lp.tile([c.DIN, c.DH], F32)

                wv = wih_sb[:].rearrange("p (c n) -> p c n", c=c.KCH)
                wsumv = whh_sb[:].rearrange("p (c n) -> p c n", c=c.KCH)

                for step in range(c.T):
                    wmat = wv if step == 0 else wsumv
                    gp = ps_ls.tile([c.GSL, 4], F32, tag="gates")
                    for g in range(4):
                        for kc in range(c.KCH):
                            nc.tensor.matmul(
                                gp[:, g : g + 1],
                                wmat[:, kc, g * c.GSL : (g + 1) * c.GSL],
                                inp[:, kc : kc + 1],
                                start=(kc == 0),
                                stop=(kc == c.KCH - 1),
                            )
                    if step == 0:
                        # wsum = wih + whh (for steps 2..T), overwrite whh
                        nc.vector.tensor_tensor(whh_sb[:], wih_sb[:], whh_sb[:],
                                                op=mybir.AluOpType.add)
                    nc.vector.tensor_tensor(gsb[:], gp[:], bsum[:],
                                            op=mybir.AluOpType.add)
                    Sig = mybir.ActivationFunctionType.Sigmoid
                    Tanh = mybir.ActivationFunctionType.Tanh
                    nc.scalar.activation(ifgo[:, 0:1], gsb[:, 0:1], Sig)
                    nc.scalar.activation(ifgo[:, 1:2], gsb[:, 1:2], Sig)
                    nc.scalar.activation(ifgo[:, 2:3], gsb[:, 2:3], Tanh)
                    nc.scalar.activation(ifgo[:, 3:4], gsb[:, 3:4], Sig)
                    # c' = f*c + i*g ; h' = o * tanh(c')
                    nc.vector.tensor_tensor(tmp[:, 0:1], ifgo[:, 0:1], ifgo[:, 2:3],
                                            op=mybir.AluOpType.mult)
                    if step == 0:
                        nc.vector.tensor_copy(cstate[:], tmp[:, 0:1])
                    else:
                        nc.vector.tensor_tensor(tmp[:, 1:2], ifgo[:, 1:2], cstate[:],
                                                op=mybir.AluOpType.mult)
                        nc.vector.tensor_tensor(cstate[:], tmp[:, 0:1], tmp[:, 1:2],
                                                op=mybir.AluOpType.add)
                    nc.scalar.activation(tmp[:, 0:1], cstate[:], Tanh)
                    h2 = tmp[:, 1:2]
                    nc.vector.tensor_tensor(h2, ifgo[:, 3:4], tmp[:, 0:1],
                                            op=mybir.AluOpType.mult)
                    # allgather h2 -> full h
                    nc.gpsimd.dma_start(hb_in[:, :], h2)
                    nc.gpsimd.collective_compute(
                        "AllGather", mybir.AluOpType.bypass,
                        replica_groups=groups,
                        ins=[hb_in[:, :].opt()],
                        outs=[hb_out[:, :].opt()],
                    )
                    if step < c.T - 1:
                        nc.sync.dma_start(
                            inpf[:], hb_out[:, 0].rearrange("(c p) -> p c", p=128))
                        nc.vector.tensor_copy(inp[:], inpf[:])
                    else:
                        nc.sync.dma_start(
                            Wf[:], hb_out[:, 0].rearrange("(a b) -> a b", a=c.DIN))
                        nc.vector.tensor_copy(W_sb[:], Wf[:])

            # ---------- phase B: xwd = dinv * (x @ W) ----------
            with (
                tc.tile_pool(name="xw", bufs=3) as xp,
                tc.tile_pool(name="psum_xw", bufs=4, space="PSUM") as ps_xw,
            ):
                xT_sb = xp.tile([c.DIN, c.SHP], BF16, tag="xT")
                nc.sync.dma_start(xT_sb[:], xT_h[:])
                for t in range(c.TILES):
                    pxw = ps_xw.tile([128, c.DH], F32, tag="pxw")
                    nc.tensor.matmul(pxw[:], xT_sb[:, t * 128 : (t + 1) * 128],
                                     W_sb[:], start=True, stop=True)
                    nc.vector.tensor_scalar(
                        xwd_sb[:, t, :], pxw[:], dinv_sb[:, t : t + 1], None,
                        op0=mybir.AluOpType.mult,
                    )
                    nc.sync.dma_start(
                        xwd_own[t * 128 : (t + 1) * 128, 0 : c.DH],
                        xwd_sb[:, t, :],
                    )

            tc.strict_bb_all_engine_barrier()
            nc.gpsimd.collective_compute(
                "AllGather", mybir.AluOpType.bypass,
                replica_groups=groups,
                ins=[xwd_own[:, :].opt()],
                outs=[xwd_full[:, :].opt()],
            )
            tc.strict_bb_all_engine_barrier()

            # ---------- idx planes: replicate 16-row master to 128 partitions ----
            with tc.tile_pool(name="planes", bufs=1) as plp:
                up = plp.tile([128, TOT // 16], I16)
                vp = plp.tile([128, TOT // 16], I16)
                for g in range(8):
                    nc.sync.dma_start(up[16 * g : 16 * (g + 1), :], uidx_h[:, :])
                    nc.sync.dma_start(vp[16 * g : 16 * (g + 1), :], vsidx_h[:, :])

                # ---------- phase 1: gather msgs + scatter-add ----------
                with tc.tile_pool(name="p1", bufs=3) as p1:
                    for bb, coff, clen, plist in struct["pieces"]:
                        msg = p1.tile([128, c.CHUNK // 128, c.ROW], F32, tag="msg")
                        nc.gpsimd.dma_gather(
                            msg[:, : clen // 128, :],
                            xwd_full[bb * 32768 :, :],
                            up[:, coff // 16 : (coff + clen) // 16],
                            clen, clen, c.ROW, single_packet=False,
                        )
                        for po, pl, chain in plist:
                            nc.gpsimd.dma_scatter_add(
                                aggs[chain][:, :],
                                msg[:, po // 128 : (po + pl) // 128, :],
                                vp[:, (coff + po) // 16 : (coff + po + pl) // 16],
                                pl, pl, c.ROW, single_packet=False,
                            )

                tc.strict_bb_all_engine_barrier()

                # ---------- phase 2: emb, uv tables ----------
                with (
                    tc.tile_pool(name="p2", bufs=3) as p2,
                    tc.tile_pool(name="psum_t", bufs=2, space="PSUM") as ps_t,
                    tc.tile_pool(name="psum_uv", bufs=2, space="PSUM") as ps_uv,
                ):
                    for t in range(c.TILES):
                        r0, r1 = t * 128, (t + 1) * 128
                        ag = [p2.tile([128, c.ROW], F32, tag=f"ag{i}", name=f"ag{i}")
                              for i in range(c.CCH)]
                        for i in range(c.CCH):
                            nc.sync.dma_start(ag[i][:], aggs[i][r0:r1, :])
                        s0 = p2.tile([128, c.DH], F32, tag="s0")
                        s1 = p2.tile([128, c.DH], F32, tag="s1")
                        nc.vector.tensor_tensor(s0[:], ag[0][:, : c.DH], ag[1][:, : c.DH],
                                                op=mybir.AluOpType.add)
                        nc.vector.tensor_tensor(s1[:], ag[2][:, : c.DH], ag[3][:, : c.DH],
                                                op=mybir.AluOpType.add)
                        nc.vector.tensor_tensor(s0[:], s0[:], s1[:],
                                                op=mybir.AluOpType.add)
                        nc.vector.tensor_tensor(s0[:], s0[:], xwd_sb[:, t, :],
                                                op=mybir.AluOpType.add)
                        emb = p2.tile([128, c.DH], F32, tag="emb")
                        nc.scalar.activation(emb[:], s0[:],
                                             mybir.ActivationFunctionType.Relu,
                                             scale=dinv_sb[:, t : t + 1])
                        pt = ps_t.tile([c.DH, 128], F32, tag="pt")
                        nc.tensor.transpose(pt[:], emb[:], ident[:])
                        embT = p2.tile([c.DH, 128], F32, tag="embT")
                        nc.vector.tensor_copy(embT[:], pt[:])
                        puv = ps_uv.tile([128, 2 * c.DH], F32, tag="puv")
                        nc.tensor.matmul(puv[:], embT[:], w1ab_sb[:],
                                         start=True, stop=True)
                        uvt = p2.tile([128, c.ROW], F32, tag="uvt")
                        nc.vector.tensor_copy(uvt[:, : 2 * c.DH], puv[:])
                        nc.sync.dma_start(uv_own[r0:r1, :], uvt[:])
                        nc.sync.dma_start(uv_shard[r0:r1, :], uvt[:])

                tc.strict_bb_all_engine_barrier()
                nc.gpsimd.collective_compute(
                    "AllGather", mybir.AluOpType.bypass,
                    replica_groups=groups,
                    ins=[uv_shard[:, :].opt()],
                    outs=[uv_full[:, :].opt()],
                )
                tc.strict_bb_all_engine_barrier()

                # ---------- phase 3: edge MLP ----------
                b2 = struct["b2"]
                with (
                    tc.tile_pool(name="p3", bufs=2) as p3,
                    tc.tile_pool(name="psum_w", bufs=4, space="PSUM") as ps_w,
                ):
                    for bb, coff, clen, _pl in struct["pieces"]:
                        ug = p3.tile([128, c.CHUNK // 128, c.ROW], F32, tag="ug")
                        vg = p3.tile([128, c.CHUNK // 128, c.ROW], F32, tag="vg")
                        nc.gpsimd.dma_gather(
                            ug[:, : clen // 128, :], uv_full[bb * 32768 :, :],
                            up[:, coff // 16 : (coff + clen) // 16],
                            clen, clen, c.ROW, single_packet=False,
                        )
                        nc.gpsimd.dma_gather(
                            vg[:, : clen // 128, :], uv_own[:, :],
                            vp[:, coff // 16 : (coff + clen) // 16],
                            clen, clen, c.ROW, single_packet=False,
                        )
                        eat = p3.tile([c.EF + 1, c.CHUNK], BF16, tag="eat")
                        nc.sync.dma_start(eat[:, :clen],
                                          eaT_h[:, coff : coff + clen])
                        lg = p3.tile([128, c.CHUNK // 128], F32, tag="lg")
                        ngrp = -(-clen // 2048)
                        for g in range(ngrp):
                            e0 = g * 2048
                            gl = min(2048, clen - e0)               # multiple of 128
                            nbk = gl // 128
                            pw = ps_w.tile([128, 512], F32, tag="pw")
                            for e in range(nbk):
                                nc.tensor.matmul(
                                    pw[:, e * c.DH : (e + 1) * c.DH],
                                    eat[:, e0 + e * 128 : e0 + (e + 1) * 128],
                                    w1c_sb[:], start=True, stop=True,
                                )
                            z = p3.tile([128, 16, c.DH], F32, tag="z")
                            blk = slice(e0 // 128, e0 // 128 + nbk)
                            nc.vector.tensor_tensor(
                                z[:, :nbk, :], ug[:, blk, : c.DH],
                                vg[:, blk, c.DH : 2 * c.DH], op=mybir.AluOpType.add,
                            )
                            nc.vector.tensor_tensor(
                                z[:].rearrange("p a b -> p (a b)")[:, : nbk * c.DH],
                                z[:].rearrange("p a b -> p (a b)")[:, : nbk * c.DH],
                                pw[:, : nbk * c.DH],
                                op=mybir.AluOpType.add,
                            )
                            nc.scalar.activation(
                                z[:, :nbk, :], z[:, :nbk, :],
                                mybir.ActivationFunctionType.Relu,
                            )
                            nc.vector.tensor_tensor(
                                z[:, :nbk, :], z[:, :nbk, :],
                                w2_sb[:].rearrange("p (a b) -> p a b", b=c.DH)[:, :nbk, :],
                                op=mybir.AluOpType.mult,
                            )
                            nc.vector.tensor_reduce(
                                lg[:, blk], z[:, :nbk, :],
                                axis=mybir.AxisListType.X, op=mybir.AluOpType.add,
                            )
                        if b2 != 0.0:
                            nc.vector.tensor_scalar_add(lg[:, : clen // 128],
                                                        lg[:, : clen // 128], b2)
                        nc.sync.dma_start(
                            logits_h[:, coff // 128 : (coff + clen) // 128],
                            lg[:, : clen // 128],
                        )

    nc.compile()
    return nc


_BUILD_CACHE = {}


def _kernel_impl(inputs, cfg):
    in_maps, struct, orig_all = host_prep(inputs, cfg)
    key = (cfg.N, cfg.E, struct["TOT"], str(struct["pieces"]), struct["b2"])
    if key not in _BUILD_CACHE:
        _BUILD_CACHE.clear()
        _BUILD_CACHE[key] = build(cfg, struct)
    nc = _BUILD_CACHE[key]
    res = run_bass_kernel_spmd(nc, in_maps, list(range(cfg.NC)))
    out = np.empty(cfg.E, np.float32)
    flat = np.concatenate([res.results[k]["logits"].T.reshape(-1)
                           for k in range(cfg.NC)])
    valid = orig_all >= 0
    out[orig_all[valid]] = flat[valid]
    return out


def kernel(**inputs):
    cfg = CFG(N=100000, E=1_600_000, T=5, DIN=32, DH=32, EF=16)
    return _kernel_impl(inputs, cfg)
